# revision 1
# baseline (speedup 1.0000x reference)
"""Trainium2 Bass kernel for nn_CoupledAttention (sparse_attention).

Strategy (data-parallel over batch N=8, one batch element per core):
  - All attention biases (temporal MLP bias + hop MLP bias) are computed on
    host (tiny MLPs) and folded into the QK^T matmul as extra contraction
    rows (K = 32 qk + 25 hop-onehot + 64 frame-onehot = 121), so the PE
    produces fully-biased logits directly:  S^T[tk, tq] (pre-scaled by
    SCALE/2; the exp applies scale=2).
  - Softmax runs without max-subtraction (logits are provably tiny) in the
    S^T orientation; denominators are obtained by appending a ones-column to
    the value matrix in the PV matmul, so the same matmul emits both
    unnormalized context and row sums.
  - The relational block-diagonal term  einsum('hpq,nhfqc->nhfpc', outer, v)
    is computed by per-frame-group (4 frames = 100 tokens) matmuls against a
    constant block-diagonal [100,100] matrix.
  - Final:  out = (alpha*ctxU/d + rel) @ proj_w.T + proj_b, computed as
    Y^T = projT.T @ final^T on the PE.

Layout notes: tokens padded 1600 -> 1664 = 13*128 chunks; padded x columns
are zero so padded keys contribute exp(0)=1 * v=0 and ones-col=0 (no effect).
tq is processed in two panels (800, 864) to fit PSUM (8 banks).
"""

import numpy as np

H = 6
P = 25
F = 64
T = 1600
TPAD = 1664
DIM = 192
HD = 32
NB = 8
SCALE = HD ** -0.5
S2 = SCALE * 0.5
NCHUNK = TPAD // 128  # 13
KAUG = 89   # 25 hop onehot + 64 frame onehot
KFULL = 121
PAIRS = [(0, 1), (2, 3), (4, 5)]
# (col_start, width) tq panels: each <= 512 (one PSUM bank per matmul) and a
# multiple of 100 (rel chunk granularity). Padded cols 1600:1664 are never
# touched by attention (only by the q/k/v projections).
PANELS = [(0, 500), (500, 500), (1000, 500), (1500, 100)]

_CACHE = {}

# fused-exp polynomial (p(w)^2 ~ exp(2w), |w| <= 0.42): minimax-fitted
C3V, C2V = 0.16654227731456347, 0.50569975039442
# chunks handled by the DVE-path fused exp (rest go to ScalarE exp)
DVE_CHUNKS = frozenset({8, 9, 10, 11, 12})


def _expb_ref(in0, in1, s0, s1, imm2):
    a = np.asarray(in0, np.float32)
    if in1 is not None:
        a = a + np.asarray(in1, np.float32).reshape(a.shape)
    p = ((s0 * a + s1) * a + 1.0) * a + 1.0
    return p * p


def _register_expb():
    """Register the fused exp DveOp (idempotent; runtime OPS append)."""
    from concourse.dve_spec import Spec, Src0, Src1, C0, C1, One, lower, sq
    from concourse.dve_uop import DveOpSpec
    import concourse.dve_ops as dmod
    from concourse.dve_ops import DveOp, OPS
    for op in OPS:
        if op.name == "EXPB_ANT":
            return op
    w = Src0 + Src1
    spec = Spec(body=sq(((C0 * w + C1) * w + One) * w + One), reference=_expb_ref)
    shas = {}
    for ver in ("v3", "v4"):
        try:
            s = DveOpSpec(name="EXPB_ANT", opcode=1, uops=lower(spec, ver=ver),
                          rd1_en=True)
            shas[ver] = s.sha(ver)
        except Exception:
            pass
    op = DveOp("EXPB_ANT", spec, subdim=False, uops_sha=shas)
    OPS.append(op)
    dmod._SUB_OPCODE_FOR_NAME[op.name] = dmod._CUSTOM_DVE_ROW_BASE + len(OPS) - 1
    dmod.CUSTOM_DVE_SPECS[op.name] = spec
    return op


def _f32(x):
    return np.ascontiguousarray(x, dtype=np.float32)


def _bf16(x):
    import ml_dtypes
    return np.ascontiguousarray(np.asarray(x, dtype=np.float32).astype(ml_dtypes.bfloat16))


def _host_prep(inputs):
    """Compute bias tables and all device input arrays on host (numpy f32)."""
    x = _f32(inputs["x"])              # (8, 1600, 192)
    qkv_w = _f32(inputs["qkv_w"])      # (576, 192)
    proj_w = _f32(inputs["proj_w"])    # (192, 192)
    proj_b = _f32(inputs["proj_b"])    # (192,)
    t_w1 = _f32(inputs["t_w1"]); t_b1 = _f32(inputs["t_b1"])
    t_w2 = _f32(inputs["t_w2"]); t_b2 = _f32(inputs["t_b2"])
    h_w1 = _f32(inputs["h_w1"]); h_b1 = _f32(inputs["h_b1"])
    h_w2 = _f32(inputs["h_w2"]); h_b2 = _f32(inputs["h_b2"])
    outer = _f32(inputs["outer"])      # (H, P, P)
    alpha = float(np.asarray(inputs["alpha"]).reshape(-1)[0])
    hop = np.asarray(inputs["hop"])    # (P, P) int32

    # --- bias tables (exactly as in reference, f32) ---
    rel = (np.arange(2 * F - 1, dtype=np.float32) - (F - 1))[:, None]   # (127,1)
    tab = np.maximum(rel @ t_w1.T + t_b1, 0.0) @ t_w2.T + t_b2          # (127, H)
    hf = hop.astype(np.float32).reshape(-1, 1)
    hb = (np.maximum(hf @ h_w1.T + h_b1, 0.0) @ h_w2.T + h_b2).reshape(P, P, H)

    tq = np.arange(TPAD)
    fidx = tq // P          # frame index (0..63 valid; >=64 for pads)
    pidx = tq % P

    # --- augmented K rows ---
    # kaug[a, tk] = [p(tk)==a] (25 rows), then [f(tk)==m] (64 rows); pads zero
    kaug = np.zeros((KAUG, TPAD), np.float32)
    for a in range(P):
        kaug[a, :T] = (pidx[:T] == a)
    for m in range(F):
        kaug[P + m, :T] = (fidx[:T] == m)

    # qaug[h, a, tq] = S2*hb[p(tq), a, h];  qaug[h, 25+m, tq] = S2*tab[f(tq)-m+63, h]
    qaug = np.zeros((H, KAUG, TPAD), np.float32)
    for h in range(H):
        qaug[h, :P, :T] = S2 * hb[pidx[:T], :, h].T                 # (25, 1600)
        dmat = fidx[:T][None, :] - np.arange(F)[:, None] + (F - 1)  # (64, 1600)
        qaug[h, P:, :T] = S2 * tab[dmat, h]

    # tabpat[h, c, p, i] = S2*tab[i - f(128c+p) + 63, h] — the Src1 pattern
    # tiles for the DVE-path fused exp (temporal bias via repeat-AP)
    tabpat = np.zeros((H, NCHUNK, 128, F), np.float32)
    fgrid = fidx.reshape(NCHUNK, 128)                    # frame of each (c, p)
    for h in range(H):
        for c in range(NCHUNK):
            idx = np.arange(F)[None, :] - fgrid[c][:, None] + (F - 1)  # (128, 64)
            valid = fgrid[c][:, None] < F
            tabpat[h, c] = np.where(valid, S2 * tab[np.clip(idx, 0, 2*F-2), h], 0.0)

    # --- projection weights (transposed, padded) ---
    # wq/wk: lhsT [e, c] chunks; M padded to 256 (cols 192:256 zero)
    def wchunks(w, scale):
        wt = np.zeros((DIM, 256), np.float32)
        wt[:, :DIM] = scale * w.T          # [e, c]
        a = np.zeros((128, 256), np.float32); a[:, :] = wt[:128]
        b = np.zeros((128, 256), np.float32); b[:64, :] = wt[128:]
        return a, b

    wq_a, wq_b = wchunks(qkv_w[0:DIM], S2)
    wk_a, wk_b = wchunks(qkv_w[DIM:2 * DIM], 1.0)
    # wv: rhs [e, c] (N=192), v scaled by alpha
    vscale = alpha if alpha != 0.0 else 1.0
    wvt = vscale * qkv_w[2 * DIM:3 * DIM].T     # (192, 192) [e, c]
    wv_a = np.zeros((128, DIM), np.float32); wv_a[:, :] = wvt[:128]
    wv_b = np.zeros((128, DIM), np.float32); wv_b[:64, :] = wvt[128:]

    # --- rel block-diagonal matrix (shared across heads; compensate alpha) ---
    out0 = outer[0]
    oscale = (1.0 / alpha) if alpha != 0.0 else 1.0
    oblk = np.zeros((100, 100), np.float32)
    for b in range(4):
        # rhs[j=tk_local, n=tq_local] = outer[p(tq), p(tk)]
        oblk[b * P:(b + 1) * P, b * P:(b + 1) * P] = oscale * out0.T
    head_indep = all(np.allclose(outer[0], outer[h]) for h in range(H))

    # --- out projection ---
    pt = np.zeros((DIM, 256), np.float32)
    pt[:, :DIM] = proj_w.T                      # [e, d]
    proj_a = np.zeros((128, 256), np.float32); proj_a[:, :] = pt[:128]
    proj_b_chunk = np.zeros((128, 256), np.float32); proj_b_chunk[:64, :] = pt[128:]
    pb_a = np.zeros((128, 1), np.float32); pb_a[:, 0] = proj_b[:128]
    pb_b = np.zeros((128, 1), np.float32); pb_b[:64, 0] = proj_b[128:]

    common = {
        "kaug": _bf16(kaug),
        "qaug": _bf16(qaug),
        "tabpat": _bf16(tabpat),
        "wq_a": _bf16(wq_a), "wq_b": _bf16(wq_b),
        "wk_a": _bf16(wk_a), "wk_b": _bf16(wk_b),
        "wv_a": _bf16(wv_a), "wv_b": _bf16(wv_b),
        "oblk": _bf16(oblk),
        "projt_a": _bf16(proj_a), "projt_b": _bf16(proj_b_chunk),
        "pb_a": _f32(pb_a), "pb_b": _f32(pb_b),
    }
    # per-core x^T padded
    xts = []
    for n in range(NB):
        xt = np.zeros((DIM, TPAD), np.float32)
        xt[:, :T] = x[n].T
        xa = np.zeros((128, TPAD), np.float32); xa[:, :] = xt[:128]
        xb = np.zeros((128, TPAD), np.float32); xb[:64, :] = xt[128:]
        xts.append((_bf16(xa), _bf16(xb)))
    return common, xts, alpha, head_indep


def _build_program(alpha, loop_n=None):
    """Emit the Bass/Tile program (data independent; alpha affects a branch)."""
    from contextlib import ExitStack
    import concourse.bass as bass
    import concourse.bacc as bacc
    import concourse.tile as tile
    from concourse import mybir

    BF = mybir.dt.bfloat16
    FP = mybir.dt.float32
    EXP = mybir.ActivationFunctionType.Exp
    IDENT = mybir.ActivationFunctionType.Identity

    EXPB = _register_expb()
    nc = bacc.Bacc("TRN2", target_bir_lowering=False, debug=False,
                   enable_asserts=False)

    def din(name, shape, dt=BF):
        return nc.dram_tensor(name, list(shape), dt, kind="ExternalInput").ap()

    d_xa = din("xt_a", (128, TPAD)); d_xb = din("xt_b", (128, TPAD))
    d_kaug = din("kaug", (KAUG, TPAD))
    d_qaug = din("qaug", (H, KAUG, TPAD))
    d_wqa = din("wq_a", (128, 256)); d_wqb = din("wq_b", (128, 256))
    d_wka = din("wk_a", (128, 256)); d_wkb = din("wk_b", (128, 256))
    d_wva = din("wv_a", (128, DIM)); d_wvb = din("wv_b", (128, DIM))
    d_oblk = din("oblk", (100, 100))
    d_tabpat = din("tabpat", (H, NCHUNK, 128, F))
    d_pta = din("projt_a", (128, 256)); d_ptb = din("projt_b", (128, 256))
    d_pba = din("pb_a", (128, 1), FP); d_pbb = din("pb_b", (128, 1), FP)
    d_ya = nc.dram_tensor("y_a", [128, T], FP, kind="ExternalOutput").ap()
    d_yb = nc.dram_tensor("y_b", [64, T], FP, kind="ExternalOutput").ap()

    # projection panels cover the padded token range, one PSUM bank each
    PPANELS = [(0, 512), (512, 512), (1024, 512), (1536, 128)]

    with tile.TileContext(nc) as tc, ExitStack() as ctx:
        singles = ctx.enter_context(tc.tile_pool(name="singles", bufs=1))
        psum = ctx.enter_context(tc.tile_pool(name="psum", bufs=1, space="PSUM"))
        ppool = ctx.enter_context(tc.tile_pool(name="ppool", bufs=28))
        cpool = ctx.enter_context(tc.tile_pool(name="cpool", bufs=2))
        dpool = ctx.enter_context(tc.tile_pool(name="dpool", bufs=2, space="DRAM"))

        if loop_n is not None:
            # benchmarking mode: run the whole body loop_n times on-device
            ctx.enter_context(tc.For_i(0, loop_n, 1))

        # ---- load constants ----
        xa = singles.tile([128, TPAD], BF); nc.gpsimd.dma_start(out=xa, in_=d_xa)
        xb = singles.tile([128, TPAD], BF); nc.gpsimd.dma_start(out=xb, in_=d_xb)
        wqa = singles.tile([128, 256], BF); nc.gpsimd.dma_start(out=wqa, in_=d_wqa)
        wqb = singles.tile([128, 256], BF); nc.gpsimd.dma_start(out=wqb, in_=d_wqb)
        wka = singles.tile([128, 256], BF); nc.gpsimd.dma_start(out=wka, in_=d_wka)
        wkb = singles.tile([128, 256], BF); nc.gpsimd.dma_start(out=wkb, in_=d_wkb)
        wva = singles.tile([128, DIM], BF); nc.gpsimd.dma_start(out=wva, in_=d_wva)
        wvb = singles.tile([128, DIM], BF); nc.gpsimd.dma_start(out=wvb, in_=d_wvb)
        oblkt = singles.tile([100, 100], BF); nc.gpsimd.dma_start(out=oblkt, in_=d_oblk)
        pta = singles.tile([128, 256], BF); nc.gpsimd.dma_start(out=pta, in_=d_pta)
        ptb = singles.tile([128, 256], BF); nc.gpsimd.dma_start(out=ptb, in_=d_ptb)
        pba = singles.tile([128, 1], FP); nc.gpsimd.dma_start(out=pba, in_=d_pba)
        pbb = singles.tile([128, 1], FP); nc.gpsimd.dma_start(out=pbb, in_=d_pbb)

        # ---- staging + per-head tiles ----
        qt4 = singles.tile([128, TPAD], BF)   # q^T heads 0..3 (rows 32h..)
        qt2 = singles.tile([64, TPAD], BF)    # heads 4,5
        kt4 = singles.tile([128, TPAD], BF)
        kt2 = singles.tile([64, TPAD], BF)
        vall = singles.tile([128, NCHUNK, H, 33], BF)
        kfull = [singles.tile([128, TPAD], BF, name=f"kfull{h}") for h in range(H)]
        qfull = [singles.tile([128, TPAD], BF, name=f"qfull{h}") for h in range(H)]
        vrelp = [singles.tile([128, 16, 66], BF, name=f"vrelp{i}") for i in range(3)]
        outta = singles.tile([128, T], BF)
        outtb = singles.tile([128, T], BF)
        nc.vector.memset(outtb, 0.0)  # rows 64:128 must be zero for K=128 matmul
        # zero operand for psum accumulation-group "openers" (K=1 matmul that
        # writes zeros to all 128 partitions of a bank with start=True)
        zrow = singles.tile([1, 512], BF)
        nc.vector.memset(zrow, 0.0)
        zcol = singles.tile([1, 128], BF)
        nc.vector.memset(zcol, 0.0)

        # ---- q^T / k^T projections: out[c, t] = w.T @ x^T ----
        for (wa, wb, st4, st2) in ((wqa, wqb, qt4, qt2), (wka, wkb, kt4, kt2)):
            for mset in range(2):
                for (c0, w) in PPANELS:
                    ps = psum.tile([128, 512], FP, tag="s", bufs=4, name="ps_proj")
                    nc.tensor.matmul(ps[:, 0:w],
                                     wa[:, mset * 128:mset * 128 + 128],
                                     xa[:, c0:c0 + w], start=True, stop=False)
                    nc.tensor.matmul(ps[:, 0:w],
                                     wb[:, mset * 128:mset * 128 + 128],
                                     xb[:, c0:c0 + w], start=False, stop=True)
                    if mset == 0:
                        nc.scalar.copy(st4[:, c0:c0 + w], ps[:, 0:w])
                    else:
                        nc.scalar.copy(st2[:, c0:c0 + w], ps[0:64, 0:w])

        # ---- v projection (token-major, alpha-scaled): v[t, c] ----
        for c in range(NCHUNK):
            ps = psum.tile([128, DIM], FP, tag="s", bufs=4, name="ps_v")
            nc.tensor.matmul(ps, xa[:, c * 128:(c + 1) * 128], wva,
                             start=True, stop=False)
            nc.tensor.matmul(ps, xb[:, c * 128:(c + 1) * 128], wvb,
                             start=False, stop=True)
            nc.scalar.copy(vall[:, c, :, 0:32], ps)
        # ones column (zero for padded tokens)
        for c in range(NCHUNK):
            rows = 64 if c == NCHUNK - 1 else 128
            nc.vector.memset(vall[0:rows, c, :, 32:33], 1.0)
            if rows < 128:
                nc.vector.memset(vall[rows:128, c, :, 32:33], 0.0)

        # ---- assemble per-head augmented q/k tiles ----
        # even heads: rows [k(0:32) | hopOH(32:57) | tempOH(57:121)]
        # odd heads:  rows [tempOH(0:64) | k(64:96) | hopOH(96:121)]
        # (odd layout keeps k+hop contiguous at base 64 for the row-tiled
        #  DVE-path QK; the K=121 contraction order is irrelevant as long as
        #  k-side and q-side rows pair up)
        for h in range(H):
            ksrc = kt4[32 * h:32 * h + 32, :] if h < 4 else \
                   kt2[32 * (h - 4):32 * (h - 4) + 32, :]
            qsrc = qt4[32 * h:32 * h + 32, :] if h < 4 else \
                   qt2[32 * (h - 4):32 * (h - 4) + 32, :]
            if h % 2 == 0:
                nc.gpsimd.dma_start(out=kfull[h][0:32, :], in_=ksrc)
                nc.gpsimd.dma_start(out=qfull[h][0:32, :], in_=qsrc)
                nc.gpsimd.dma_start(out=kfull[h][32:32 + KAUG, :], in_=d_kaug)
                nc.gpsimd.dma_start(out=qfull[h][32:32 + KAUG, :], in_=d_qaug[h])
            else:
                nc.gpsimd.dma_start(out=kfull[h][0:64, :], in_=d_kaug[P:KAUG, :])
                nc.gpsimd.dma_start(out=qfull[h][0:64, :], in_=d_qaug[h, P:KAUG, :])
                nc.gpsimd.dma_start(out=kfull[h][64:96, :], in_=ksrc)
                nc.gpsimd.dma_start(out=qfull[h][64:96, :], in_=qsrc)
                nc.gpsimd.dma_start(out=kfull[h][96:121, :], in_=d_kaug[0:P, :])
                nc.gpsimd.dma_start(out=qfull[h][96:121, :], in_=d_qaug[h, 0:P, :])
        tabsb = [singles.tile([128, NCHUNK, F], BF, name=f"tabsb{h}") for h in range(H)]
        for h in range(H):
            nc.gpsimd.dma_start(out=tabsb[h],
                                in_=d_tabpat[h].rearrange("c p i -> p c i"))

        # ---- vrelp: v regrouped into 100-token (4-frame) chunks ----
        for ip, (h0, h1) in enumerate(PAIRS):
            for g in range(16):
                t0 = 100 * g
                c, p0 = divmod(t0, 128)
                n1 = min(128 - p0, 100)
                nc.gpsimd.dma_start(out=vrelp[ip][0:n1, g, :],
                                  in_=vall[p0:p0 + n1, c, h0:h0 + 2, :])
                if n1 < 100:
                    nc.gpsimd.dma_start(out=vrelp[ip][n1:100, g, :],
                                      in_=vall[0:100 - n1, c + 1, h0:h0 + 2, :])

        # ---- attention blocks (skewed pipeline) ----
        blocks = [(ip, ih) for ip in range(3) for ih in range(len(PANELS))]
        state = {}

        def tab_ap(h, c, c0, w):
            # Src1 repeat-pattern: element [p, j] reads tabsb[h][p, c, (c0+j)//25]
            base = tabsb[h]
            return bass.AP(
                tensor=base.tensor,
                offset=base.offset + c * F + c0 // P,
                ap=[[base.ap[0][0], 128], [1, w // P], [0, P]])

        def emit_qk_exp(blk):
            ip, ih = blk
            h0, h1 = PAIRS[ip]
            c0, w = PANELS[ih]
            ptiles = {}
            # phase A: ScalarE-exp chunks (bias fully in-matmul, K=121)
            for c in range(NCHUNK):
                if c in DVE_CHUNKS:
                    continue
                for h in (h0, h1):
                    ps = psum.tile([128, 512], FP, tag="s", bufs=4, name="ps_s")
                    nc.tensor.matmul(ps[:, 0:w],
                                     kfull[h][0:KFULL, c * 128:(c + 1) * 128],
                                     qfull[h][0:KFULL, c0:c0 + w],
                                     start=True, stop=True)
                    pt = ppool.tile([128, 512], BF, tag="p", name="ptile")
                    nc.scalar.activation(pt[:, 0:w], ps[:, 0:w], EXP, scale=2.0)
                    ptiles[(c, h)] = pt
            # phase B: DVE-path chunks — 2-head row-tiled QK (K=57, hop
            # in-matmul) + fused poly-exp with the temporal bias via Src1
            for c in range(NCHUNK):
                if c not in DVE_CHUNKS:
                    continue
                psA = psum.tile([128, 512], FP, tag="s", bufs=4, name="ps_s")
                psB = psum.tile([128, 512], FP, tag="s", bufs=4, name="ps_s")
                nc.tensor.matmul(psA[:, 0:w],
                                 kfull[h0][0:57, c * 128:(c + 1) * 128],
                                 qfull[h0][0:57, c0:c0 + w],
                                 start=True, stop=True)
                nc.tensor.matmul(psB[:, 0:w],
                                 kfull[h1][64:121, c * 128:(c + 1) * 128],
                                 qfull[h1][64:121, c0:c0 + w],
                                 start=True, stop=True)
                for h, ps in ((h0, psA), (h1, psB)):
                    pt = ppool.tile([128, 512], BF, tag="p", name="ptile")
                    nc.vector._custom_dve(
                        EXPB,
                        out=pt[:, 0:w].rearrange("p (a b) -> p a b", b=P),
                        in0=ps[:, 0:w].rearrange("p (a b) -> p a b", b=P),
                        in1=tab_ap(h, c, c0, w),
                        s0=C3V, s1=C2V)
                    ptiles[(c, h)] = pt
            state[blk] = {"ptiles": ptiles}

        def emit_pv_rel(blk):
            ip, ih = blk
            h0, h1 = PAIRS[ip]
            c0, w = PANELS[ih]
            ptiles = state[blk]["ptiles"]
            ctx_ps = psum.tile([128, 512], FP, tag="ctx", bufs=2, name="ps_ctx")
            # PSUM accumulation-group opener: zero-write all 128 partitions of
            # the bank (K=1 matmul of zeros) so both col-tiled streams can
            # accumulate with start=False.
            nc.tensor.matmul(ctx_ps[:, 0:w], zcol, zrow[:, 0:w],
                             start=True, stop=False)
            for c in range(NCHUNK):
                for hi, h in ((0, h0), (1, h1)):
                    pt = ptiles[(c, h)]
                    nc.tensor.matmul(
                        ctx_ps[64 * hi:64 * hi + 33, 0:w],
                        vall[:, c, h, :],
                        pt[:, 0:w],
                        start=False, stop=False)
            # closer: +0 over all partitions, closes the group
            nc.tensor.matmul(ctx_ps[:, 0:w], zcol, zrow[:, 0:w],
                             start=False, stop=True)
            rel_ps = psum.tile([128, 512], FP, tag="rel", bufs=2, name="ps_rel")
            # zero-fill so the combine's full-partition read is defined
            nc.tensor.matmul(rel_ps[:, 0:w], zcol, zrow[:, 0:w],
                             start=True, stop=True)
            for g in range(w // 100):
                gg = c0 // 100 + g
                for hi in range(2):
                    nc.tensor.matmul(
                        rel_ps[64 * hi:64 * hi + 33, g * 100:(g + 1) * 100],
                        vrelp[ip][0:100, gg, 33 * hi:33 * hi + 33],
                        oblkt,
                        start=True, stop=True)
            state[blk]["ctx"] = ctx_ps
            state[blk]["rel"] = rel_ps

        def emit_combine(blk):
            ip, ih = blk
            c0, w = PANELS[ih]
            ctx_ps = state[blk]["ctx"]; rel_ps = state[blk]["rel"]
            rbsb = cpool.tile([128, 512], FP, tag="rbsb", name="rbsb")
            if alpha != 0.0:
                recipd = cpool.tile([128, 512], FP, tag="recipd", name="recipd")
                nc.vector.reciprocal(recipd[32:33, 0:w], ctx_ps[32:33, 0:w])
                nc.vector.reciprocal(recipd[96:97, 0:w], ctx_ps[96:97, 0:w])
                dscr = dpool.tile([2, 512], FP, tag="dscr", name="dscr")
                nc.sync.dma_start(out=dscr[0:1, 0:w], in_=recipd[32:33, 0:w])
                nc.sync.dma_start(out=dscr[1:2, 0:w], in_=recipd[96:97, 0:w])
                src0 = bass.AP(tensor=dscr.tensor, offset=dscr.offset,
                               ap=[[0, 64], [1, w]])
                src1 = bass.AP(tensor=dscr.tensor, offset=dscr.offset + 512,
                               ap=[[0, 64], [1, w]])
                nc.sync.dma_start(out=rbsb[0:64, 0:w], in_=src0)
                nc.sync.dma_start(out=rbsb[64:128, 0:w], in_=src1)
            else:
                nc.vector.memset(rbsb, 0.0)
            t1 = cpool.tile([128, 512], FP, tag="t1", name="t1")
            nc.vector.tensor_mul(t1[:, 0:w], ctx_ps[:, 0:w], rbsb[:, 0:w])
            pout = cpool.tile([128, 512], BF, tag="pout", name="pout")
            nc.vector.tensor_add(pout[:, 0:w], rel_ps[:, 0:w], t1[:, 0:w])
            dsts = ((outta, 64 * ip), (outta, 64 * ip + 32)) if ip < 2 else \
                   ((outtb, 0), (outtb, 32))
            for hi, (dstt, r0) in enumerate(dsts):
                nc.gpsimd.dma_start(
                    out=dstt[r0:r0 + 32, c0:c0 + w],
                    in_=pout[64 * hi:64 * hi + 32, 0:w])

        for i, blk in enumerate(blocks):
            emit_qk_exp(blk)
            if i > 0:
                emit_pv_rel(blocks[i - 1])
                emit_combine(blocks[i - 1])
        emit_pv_rel(blocks[-1])
        emit_combine(blocks[-1])

        # ---- output projection: Y^T[d, t] = projT.T @ out^T ----
        YPANELS = [(0, 512), (512, 512), (1024, 512), (1536, 64)]
        for mset in range(2):
            for (c0, w) in YPANELS:
                ps = psum.tile([128, 512], FP, tag="s", bufs=4, name="ps_y")
                nc.tensor.matmul(ps[:, 0:w],
                                 pta[:, mset * 128:mset * 128 + 128],
                                 outta[:, c0:c0 + w], start=True, stop=False)
                nc.tensor.matmul(ps[:, 0:w],
                                 ptb[:, mset * 128:mset * 128 + 128],
                                 outtb[:, c0:c0 + w], start=False, stop=True)
                ysb = cpool.tile([128, 512], FP, tag="ysb", name="ysb")
                pb = pba if mset == 0 else pbb
                nc.scalar.activation(ysb[:, 0:w], ps[:, 0:w], IDENT,
                                     bias=pb, scale=1.0)
                if mset == 0:
                    nc.gpsimd.dma_start(out=d_ya[:, c0:c0 + w], in_=ysb[:, 0:w])
                else:
                    nc.gpsimd.dma_start(out=d_yb[:, c0:c0 + w], in_=ysb[0:64, 0:w])

    nc.compile()
    return nc


def kernel(**inputs):
    common, xts, alpha, head_indep = _host_prep(inputs)
    assert head_indep, "outer must be head-independent (np.tile in reference)"

    key = ("prog", alpha == 0.0)
    if key not in _CACHE:
        _CACHE[key] = _build_program(alpha)
    nc = _CACHE[key]

    in_maps = []
    for n in range(NB):
        m = dict(common)
        m["xt_a"], m["xt_b"] = xts[n]
        in_maps.append(m)

    from concourse.bass_utils import run_bass_kernel_spmd
    res = run_bass_kernel_spmd(nc, in_maps, core_ids=list(range(NB)))
    out = np.zeros((NB, T, DIM), np.float32)
    for n in range(NB):
        ya = np.asarray(res.results[n]["y_a"], np.float32)   # (128, 1600)
        yb = np.asarray(res.results[n]["y_b"], np.float32)   # (64, 1600)
        out[n] = np.concatenate([ya, yb], axis=0).T
    return out

